# revision 2
# baseline (speedup 1.0000x reference)
"""Trainium2 Bass kernel for CMELossAngularProfileMSE_V2.

Strategy (pure data parallel over batch, 8 NeuronCores):
  - Host downcasts mask_pred to bf16 (quantization error on a
    2048-element radial mean is ~1e-5 relative -- far below the 2e-2
    gate) and ships per-core tiles [16, 128, 5760]: partition p holds
    r in [16p, 16p+16), free dim = 16 contiguous theta-slices of 360.
  - Per sample, two half DMAs land [128, 2880] bf16 each; one 1440-wide
    DVE add folds each half's 8 slices -> 4 (bf16 adds run the 2x DVE
    mode: ~0.52 ns/elem), leaving 8 slices per sample.
  - 8 bf16 one-hot matmuls per sample reduce over partitions into row b
    of a PSUM tile [16, 360], accumulating the raw radial sums
    S[b, theta] across all samples (~300 ns each sustained).
  - Host precomputes T' = R*T and w' = w/R^2 (exact power-of-two
    scalings of the Gaussian target / distance weight derived from
    theta_min/theta_max), so the device epilogue is just
    sum_theta((S - T')^2 * w') per sample -> out [16, 1], all on DVE.
  - Host: loss = sum(all per-sample sums) / (360 * 128).
"""
import numpy as np
import ml_dtypes

import concourse.bacc as bacc
import concourse.tile as tile
from concourse import mybir
from concourse.bass_utils import run_bass_kernel_spmd

F32 = mybir.dt.float32
BF16 = mybir.dt.bfloat16

N_CORES = 8
B = 128            # full batch
BS = B // N_CORES  # samples per core (16)
R = 2048
TH = 360
Q = 16             # theta-slices per partition (2048 = 128 * 16)
HALF = 8 * TH      # 2880
QTR = 4 * TH       # 1440
SIGMA = 10.0
ALPHA_WEIGHT = 2.0
LAMBDA_ANG = 1.0


def _build_nc():
    nc = bacc.Bacc("TRN2", target_bir_lowering=False, debug=False)
    x = nc.dram_tensor("x", [BS, 128, Q * TH], BF16, kind="ExternalInput").ap()
    oh = nc.dram_tensor("oh", [128, BS * BS], BF16, kind="ExternalInput").ap()
    tw = nc.dram_tensor("tw", [BS, 2 * TH], F32, kind="ExternalInput").ap()
    out = nc.dram_tensor("out", [BS, 1], F32, kind="ExternalOutput").ap()

    from contextlib import ExitStack
    with tile.TileContext(nc) as tc, ExitStack() as ctx:
        consts = ctx.enter_context(tc.tile_pool(name="consts", bufs=1))
        inp = ctx.enter_context(tc.tile_pool(name="inp", bufs=6))
        psum = ctx.enter_context(tc.tile_pool(name="psum", bufs=1, space="PSUM"))
        small = ctx.enter_context(tc.tile_pool(name="small", bufs=1))

        # one-hot stationaries (O[:, b, j] = 1 iff j == b) and the
        # epilogue constants arrive via two small DMAs, keeping engines
        # free of memset preamble.
        O = consts.tile([128, BS, BS], BF16)
        t16w16 = small.tile([BS, 2, TH], F32)
        t16 = t16w16[:, 0, :]
        w16 = t16w16[:, 1, :]

        ps = psum.tile([BS, TH], F32)
        for b in range(BS):
            xt = inp.tile([128, Q * TH], BF16)
            # stream halves; the last sample streams quarters so the
            # final fold+matmul tail after the last byte is short.
            n_chunks = 4 if b == BS - 1 else 2
            step = (Q * TH) // n_chunks
            for c in range(n_chunks):
                nc.sync.dma_start(
                    xt[:, c * step:(c + 1) * step],
                    x[b][:, c * step:(c + 1) * step],
                )
            if b == 0:
                nc.sync.dma_start(O[:], oh.rearrange("p (a b) -> p a b", a=BS))
                nc.sync.dma_start(
                    t16w16[:], tw.rearrange("b (two t) -> b two t", two=2),
                )
            if b < BS - 1:
                # per half: fold 8 slices -> 4 with one 1440-wide bf16 add,
                # then 4 one-hot matmuls accumulate into PSUM row b.
                for c in range(2):
                    base = c * HALF
                    nc.vector.tensor_add(
                        xt[:, base:base + QTR], xt[:, base:base + QTR],
                        xt[:, base + QTR:base + HALF],
                    )
                    for k in range(4):
                        s = base + k * TH
                        nc.tensor.matmul(
                            ps[:], O[:, b, :], xt[:, s:s + TH],
                            start=(b == 0 and c == 0 and k == 0), stop=False,
                        )
            else:
                # quarters: fold 4 slices -> 2 per quarter (720-wide add)
                for c in range(4):
                    base = c * QTR
                    nc.vector.tensor_add(
                        xt[:, base:base + 2 * TH], xt[:, base:base + 2 * TH],
                        xt[:, base + 2 * TH:base + 4 * TH],
                    )
                    for k in range(2):
                        s = base + k * TH
                        nc.tensor.matmul(
                            ps[:], O[:, b, :], xt[:, s:s + TH],
                            start=False, stop=(c == 3 and k == 1),
                        )

        d16 = small.tile([BS, TH], F32)
        nc.vector.scalar_tensor_tensor(
            d16[:], ps[:], 1.0, t16,
            op0=mybir.AluOpType.mult, op1=mybir.AluOpType.subtract,
        )
        sq16 = small.tile([BS, TH], F32)
        nc.vector.scalar_tensor_tensor(
            sq16[:], d16[:], 1.0, d16[:],
            op0=mybir.AluOpType.mult, op1=mybir.AluOpType.mult,
        )
        sqw16 = small.tile([BS, TH], F32)
        red = small.tile([BS, 1], F32)
        nc.vector.scalar_tensor_tensor(
            sqw16[:], sq16[:], 1.0, w16,
            op0=mybir.AluOpType.mult, op1=mybir.AluOpType.mult,
            accum_out=red[:],
        )
        nc.sync.dma_start(out[:], red[:])
    nc.compile()
    return nc


def _target_and_weight(theta_min: np.ndarray, theta_max: np.ndarray):
    """Gaussian soft target T and distance weight w, [B, TH] float32 each.

    Mirrors the reference formulas (computed in float64, cast to float32;
    differences vs the f32 jax pipeline are O(1 ulp))."""
    theta = np.arange(TH, dtype=np.float64)[None, None, :]      # [1, 1, TH]
    tmin = theta_min.astype(np.float64)[:, :, None]             # [B, K, 1]
    tmax = theta_max.astype(np.float64)[:, :, None]

    center_wrap = np.mod(0.5 * (tmin + tmax + 360.0), 360.0)
    center_t = np.where(tmin <= tmax, 0.5 * (tmin + tmax), center_wrap)
    d = np.abs(theta - center_t)
    dist_t = np.minimum(d, 360.0 - d)                           # [B, K, TH]
    T = np.clip(np.exp(-0.5 * (dist_t / SIGMA) ** 2).sum(axis=1), 0.0, 1.0)

    center_w = (tmin + np.mod(tmax - tmin, 360.0)) / 2.0
    dw = np.abs(theta - center_w)
    dist_w = np.minimum(dw, 360.0 - dw)
    w = 1.0 + ALPHA_WEIGHT * (dist_w.max(axis=1) / 180.0)       # [B, TH]

    # Feed the device T' = R*T and w' = w/R^2 (both exact scalings by
    # powers of two) so it can use the raw radial sums S instead of the
    # mean A = S/R:  ((S - R*T)^2 * w/R^2) == ((A - T)^2 * w).
    Tp = (T * np.float32(R)).astype(np.float32)
    wp = (w / np.float32(R) ** 2).astype(np.float32)
    return Tp, wp


_NC_CACHE = None


def _get_nc():
    global _NC_CACHE
    if _NC_CACHE is None:
        _NC_CACHE = _build_nc()
    return _NC_CACHE


def _run(mask_pred, theta_min, theta_max, trace=False, trace_kwargs=None,
         trace_cores=None):
    mask_pred = np.asarray(mask_pred, dtype=np.float32)
    theta_min = np.asarray(theta_min)
    theta_max = np.asarray(theta_max)
    T, w = _target_and_weight(theta_min, theta_max)
    tw_full = np.concatenate([T[:, None, :], w[:, None, :]], axis=1)  # [B,2,TH]
    tw_full = tw_full.reshape(B, 2 * TH)

    x_bf16 = mask_pred[:, 0].astype(ml_dtypes.bfloat16)  # [B, R, TH]

    oh = np.zeros((128, BS, BS), dtype=ml_dtypes.bfloat16)
    for b in range(BS):
        oh[:, b, b] = 1.0
    oh = oh.reshape(128, BS * BS)

    in_maps = []
    for i in range(N_CORES):
        sl = slice(i * BS, (i + 1) * BS)
        x_core = np.ascontiguousarray(x_bf16[sl]).reshape(BS, 128, Q * TH)
        in_maps.append({"x": x_core, "oh": oh, "tw": tw_full[sl]})

    kwargs = {}
    if trace:
        kwargs["trace"] = True
        if trace_kwargs:
            kwargs["trace_kwargs"] = trace_kwargs
        if trace_cores is not None:
            kwargs["trace_cores"] = trace_cores
    res = run_bass_kernel_spmd(_get_nc(), in_maps, core_ids=list(range(N_CORES)),
                               **kwargs)
    per_sample = np.concatenate(
        [res.results[i]["out"][:, 0] for i in range(N_CORES)]
    )
    total = per_sample.astype(np.float64).sum() / (TH * B)
    return np.float32(LAMBDA_ANG * total), res


def kernel(mask_pred: np.ndarray, theta_min: np.ndarray,
           theta_max: np.ndarray) -> np.ndarray:
    loss, _ = _run(mask_pred, theta_min, theta_max)
    return np.asarray(loss, dtype=np.float32)


# revision 3
# speedup vs baseline: 1.6220x; 1.6220x over previous
"""Trainium2 Bass kernel for CMELossAngularProfileMSE_V2.

Strategy (pure data parallel over batch, 8 NeuronCores):
  - Host downcasts mask_pred to fp8e4m3 (quantization error on a
    2048-element radial mean is ~1e-4 relative -- far below the 2e-2
    gate) and ships per-core tiles [4, 128, 4*5760]: 4 blocks of 4
    samples; within a block, partition p holds r in [16p, 16p+16) for
    each sample s at free offset s*5760, as 16 theta-slices of 360.
    23 KB contiguous DMA lines per partition keep the 16 HWDGE engines
    at peak (~26 GB/s each).
  - No radial folds at all: each of the 256 slices goes straight to a
    one-hot fp8 matmul that reduces over partitions into PSUM row b.
    Consecutive matmuls alternate between two PSUM accumulators at
    base partitions 0 and 32 (different PE column groups), which lets
    the PE pipeline them back-to-back (~75 ns/matmul sustained), so
    S[b, theta] = ps0 + ps1 raw radial sums in fp32 PSUM (exact).
  - Host precomputes T' = R*T and w' = w/R^2 (exact power-of-two
    scalings of the Gaussian target / distance weight derived from
    theta_min/theta_max), so the device epilogue is
    sum_theta(((ps0 - T') + ps1)^2 * w') per sample -> out [16, 1].
  - Host: loss = sum(all per-sample sums) / (360 * 128).
"""
import numpy as np
import ml_dtypes

import concourse.bacc as bacc
import concourse.tile as tile
from concourse import mybir
from concourse.bass_utils import run_bass_kernel_spmd

F32 = mybir.dt.float32
FP8 = mybir.dt.float8e4

N_CORES = 8
B = 128            # full batch
BS = B // N_CORES  # samples per core (16)
R = 2048
TH = 360
Q = 16             # theta-slices per partition-sample (2048 = 128 * 16)
SW = Q * TH        # 5760, one sample's free width
GB = 4             # samples per block
NBLK = BS // GB    # 4 blocks
SIGMA = 10.0
ALPHA_WEIGHT = 2.0
LAMBDA_ANG = 1.0


def _build_nc():
    nc = bacc.Bacc("TRN2", target_bir_lowering=False, debug=False)
    x = nc.dram_tensor("x", [NBLK, 128, GB * SW], FP8, kind="ExternalInput").ap()
    oh = nc.dram_tensor("oh", [128, BS * BS], FP8, kind="ExternalInput").ap()
    tw = nc.dram_tensor("tw", [BS, 2 * TH], F32, kind="ExternalInput").ap()
    out = nc.dram_tensor("out", [BS, 1], F32, kind="ExternalOutput").ap()

    from contextlib import ExitStack
    with tile.TileContext(nc) as tc, ExitStack() as ctx:
        consts = ctx.enter_context(tc.tile_pool(name="consts", bufs=1))
        inp = ctx.enter_context(tc.tile_pool(name="inp", bufs=3))
        psum = ctx.enter_context(tc.tile_pool(name="psum", bufs=1, space="PSUM"))
        small = ctx.enter_context(tc.tile_pool(name="small", bufs=1))

        O = consts.tile([128, BS, BS], FP8)
        t16w16 = small.tile([BS, 2, TH], F32)
        t16 = t16w16[:, 0, :]
        w16 = t16w16[:, 1, :]

        # two accumulators at PSUM base partitions 0 and 32: alternating
        # matmuls land in different PE column groups and pipeline 2-wide.
        ps = psum.tile([48, TH], F32)
        n_mm = 0
        last_mm = BS * Q - 1

        def slice_mm(xt, b, s, q):
            nonlocal n_mm
            g = n_mm % 2
            off = s * SW + q * TH
            nc.tensor.matmul(
                ps[32 * g:32 * g + 16, :], O[:, b, :], xt[:, off:off + TH],
                start=(n_mm < 2), stop=(n_mm >= last_mm - 1),
            )
            n_mm += 1

        for blk in range(NBLK):
            xt = inp.tile([128, GB * SW], FP8)
            if blk < NBLK - 1:
                nc.sync.dma_start(xt[:], x[blk])
                if blk == 0:
                    nc.sync.dma_start(O[:], oh.rearrange("p (a b) -> p a b", a=BS))
                    nc.sync.dma_start(
                        t16w16[:], tw.rearrange("b (two t) -> b two t", two=2),
                    )
                for s in range(GB):
                    for q in range(Q):
                        slice_mm(xt, blk * GB + s, s, q)
            else:
                # last block streams in half-sample chunks so the final
                # matmul tail after the last byte stays short
                for c in range(2 * GB):
                    s, h = c // 2, c % 2
                    base = s * SW + h * (SW // 2)
                    nc.sync.dma_start(
                        xt[:, base:base + SW // 2],
                        x[blk][:, base:base + SW // 2],
                    )
                    for q in range(h * 8, h * 8 + 8):
                        slice_mm(xt, blk * GB + s, s, q)

        # epilogue: d = (ps0 - T') + ps1 ; out_b = sum_theta(d^2 * w')
        d16 = small.tile([BS, TH], F32)
        nc.vector.scalar_tensor_tensor(
            d16[:], ps[0:BS, :], 1.0, t16,
            op0=mybir.AluOpType.mult, op1=mybir.AluOpType.subtract,
        )
        d16b = small.tile([BS, TH], F32)
        nc.vector.tensor_add(d16b[:], d16[:], ps[32:32 + BS, :])
        sq16 = small.tile([BS, TH], F32)
        nc.vector.scalar_tensor_tensor(
            sq16[:], d16b[:], 1.0, d16b[:],
            op0=mybir.AluOpType.mult, op1=mybir.AluOpType.mult,
        )
        sqw16 = small.tile([BS, TH], F32)
        red = small.tile([BS, 1], F32)
        nc.vector.scalar_tensor_tensor(
            sqw16[:], sq16[:], 1.0, w16,
            op0=mybir.AluOpType.mult, op1=mybir.AluOpType.mult,
            accum_out=red[:],
        )
        nc.sync.dma_start(out[:], red[:])
    nc.compile()
    return nc


def _target_and_weight(theta_min: np.ndarray, theta_max: np.ndarray):
    """Gaussian soft target T and distance weight w, [B, TH] float32 each.

    Mirrors the reference formulas (computed in float64, cast to float32;
    differences vs the f32 jax pipeline are O(1 ulp))."""
    theta = np.arange(TH, dtype=np.float64)[None, None, :]      # [1, 1, TH]
    tmin = theta_min.astype(np.float64)[:, :, None]             # [B, K, 1]
    tmax = theta_max.astype(np.float64)[:, :, None]

    center_wrap = np.mod(0.5 * (tmin + tmax + 360.0), 360.0)
    center_t = np.where(tmin <= tmax, 0.5 * (tmin + tmax), center_wrap)
    d = np.abs(theta - center_t)
    dist_t = np.minimum(d, 360.0 - d)                           # [B, K, TH]
    T = np.clip(np.exp(-0.5 * (dist_t / SIGMA) ** 2).sum(axis=1), 0.0, 1.0)

    center_w = (tmin + np.mod(tmax - tmin, 360.0)) / 2.0
    dw = np.abs(theta - center_w)
    dist_w = np.minimum(dw, 360.0 - dw)
    w = 1.0 + ALPHA_WEIGHT * (dist_w.max(axis=1) / 180.0)       # [B, TH]

    # Feed the device T' = R*T and w' = w/R^2 (both exact scalings by
    # powers of two) so it can use the raw radial sums S instead of the
    # mean A = S/R:  ((S - R*T)^2 * w/R^2) == ((A - T)^2 * w).
    Tp = (T * np.float32(R)).astype(np.float32)
    wp = (w / np.float32(R) ** 2).astype(np.float32)
    return Tp, wp


_NC_CACHE = None


def _get_nc():
    global _NC_CACHE
    if _NC_CACHE is None:
        _NC_CACHE = _build_nc()
    return _NC_CACHE


def _run(mask_pred, theta_min, theta_max, trace=False, trace_kwargs=None,
         trace_cores=None):
    mask_pred = np.asarray(mask_pred, dtype=np.float32)
    theta_min = np.asarray(theta_min)
    theta_max = np.asarray(theta_max)
    T, w = _target_and_weight(theta_min, theta_max)
    tw_full = np.concatenate([T[:, None, :], w[:, None, :]], axis=1)
    tw_full = tw_full.reshape(B, 2 * TH)

    x8 = mask_pred[:, 0].astype(ml_dtypes.float8_e4m3fn)  # [B, R, TH]

    oh = np.zeros((128, BS, BS), dtype=ml_dtypes.float8_e4m3fn)
    for b in range(BS):
        oh[:, b, b] = 1.0
    oh = oh.reshape(128, BS * BS)

    in_maps = []
    for i in range(N_CORES):
        sl = slice(i * BS, (i + 1) * BS)
        # [BS, R, TH] -> [NBLK, GB, 128, SW] -> [NBLK, 128, GB*SW]
        xc = x8[sl].reshape(NBLK, GB, 128, SW)
        xc = np.ascontiguousarray(xc.transpose(0, 2, 1, 3)).reshape(
            NBLK, 128, GB * SW)
        in_maps.append({"x": xc, "oh": oh, "tw": tw_full[sl]})

    kwargs = {}
    if trace:
        kwargs["trace"] = True
        if trace_kwargs:
            kwargs["trace_kwargs"] = trace_kwargs
        if trace_cores is not None:
            kwargs["trace_cores"] = trace_cores
    res = run_bass_kernel_spmd(_get_nc(), in_maps, core_ids=list(range(N_CORES)),
                               **kwargs)
    per_sample = np.concatenate(
        [res.results[i]["out"][:, 0] for i in range(N_CORES)]
    )
    total = per_sample.astype(np.float64).sum() / (TH * B)
    return np.float32(LAMBDA_ANG * total), res


def kernel(mask_pred: np.ndarray, theta_min: np.ndarray,
           theta_max: np.ndarray) -> np.ndarray:
    loss, _ = _run(mask_pred, theta_min, theta_max)
    return np.asarray(loss, dtype=np.float32)
